# revision 2
# baseline (speedup 1.0000x reference)
"""ALiBi bias kernel for Trainium2, SPMD across 8 NeuronCores.

Output: bias[h, i, j] = -slopes[h] * (j - i) if j > i else 0, for
h in [0, 16), i, j in [0, 4096).  ~1 GiB of f32, head-parallel across
8 cores (2 heads per core).

Strategy: within one head, output row i is a shifted copy of the ramp
v[d] = -slope * relu(d).  We build a host-side "skewed" table
    tbl[p, x] = -slope * relu(x - p),   p in [0,128), x in [0,4096)
so that for the 128-row output tile starting at row i0 = 128*t, the
element bias[i0 + p, i0 + x] equals tbl[p, x] exactly.  The kernel is
then pure DMA: load the two per-head tables into SBUF once, and store
each output tile as a plain SBUF->DRAM copy of a prefix of the table.
No on-device compute at all -> runs at HBM write bandwidth.

run_bass_kernel_spmd pre-zeroes ExternalOutput buffers (kernels that
don't write every element rely on that), so we only write the columns
j >= 128*t of each tile row block; everything to the left is in the
strictly-lower causal triangle and identically zero.  This halves the
written bytes (~69 MB/core instead of 128 MiB/core).

Both HWDGE rings (SP via nc.sync, Activation via nc.scalar) are used:
each ring loads one head's table and then stores half the tiles, so
the 16 SDMA engines interleave two descriptor streams and per-DMA
fixed costs overlap.
"""

import sys

if "/opt/trn_rl_repo" not in sys.path:
    sys.path.insert(0, "/opt/trn_rl_repo")

import numpy as np

import concourse.bass as bass
import concourse.mybir as mybir
from concourse.bass_utils import run_bass_kernel_spmd

N_CORES = 8
N_HEADS = 16
HPC = N_HEADS // N_CORES  # heads per core
S = 4096  # seq_len
P = 128  # SBUF partitions / rows per tile
NT = S // P  # tiles per head

_cache: dict = {}


def _build() -> bass.Bass:
    nc = bass.Bass()
    tbl_ext = nc.declare_dram_parameter(
        "tbl", [P, HPC * S], mybir.dt.float32, isOutput=False
    )
    out_ext = nc.declare_dram_parameter(
        "out", [HPC, S, S], mybir.dt.float32, isOutput=True
    )

    def store_tile(eng, sb, t):
        # one DMA covering both heads' rows [128t, 128t+128) x cols [128t, S)
        w = S - P * t
        src = sb[:, :].rearrange("p (h x) -> p h x", h=HPC)[:, :, :w]
        dst = out_ext[:, P * t : P * (t + 1), P * t : S].transpose([1, 0, 2])
        return eng.dma_start(out=dst, in_=src)

    with (
        nc.sbuf_tensor([P, HPC * S], mybir.dt.float32) as sb,
        nc.semaphore("loadA") as loadA,
        nc.semaphore("loadB") as loadB,
        nc.semaphore("storeA") as storeA,
        nc.semaphore("storeB") as storeB,
        nc.Block() as block,
    ):

        @block.sync
        def _(sync):
            sync.dma_start(out=sb[:, 0:S], in_=tbl_ext[:, 0:S]).then_inc(loadA, 16)
            sync.wait_ge(loadA, 16)
            sync.wait_ge(loadB, 16)
            n = 0
            for t in range(0, NT, 2):
                store_tile(sync, sb, t).then_inc(storeA, 16)
                n += 1
            sync.wait_ge(storeA, 16 * n)

        @block.scalar
        def _(scalar):
            scalar.dma_start(out=sb[:, S : HPC * S], in_=tbl_ext[:, S : HPC * S]).then_inc(
                loadB, 16
            )
            scalar.wait_ge(loadA, 16)
            scalar.wait_ge(loadB, 16)
            n = 0
            for t in range(1, NT, 2):
                store_tile(scalar, sb, t).then_inc(storeB, 16)
                n += 1
            scalar.wait_ge(storeB, 16 * n)

    return nc


def _get_nc() -> bass.Bass:
    if "nc" not in _cache:
        _cache["nc"] = _build()
    return _cache["nc"]


def _tables(slopes: np.ndarray) -> np.ndarray:
    """[N_HEADS, P, S] f32: tbl[h, p, x] = -slopes[h] * relu(x - p)."""
    x = np.arange(S, dtype=np.float32)[None, :]
    p = np.arange(P, dtype=np.float32)[:, None]
    base = np.maximum(x - p, np.float32(0.0))  # [P, S]
    return (-slopes.astype(np.float32)[:, None, None]) * base[None, :, :]


def kernel(slopes: np.ndarray, seq_len) -> np.ndarray:
    assert int(seq_len) == S, f"kernel hardcoded for seq_len={S}, got {seq_len}"
    slopes = np.asarray(slopes, dtype=np.float32)
    assert slopes.shape == (N_HEADS,)

    tables = _tables(slopes)  # [16, P, S]
    in_maps = []
    for c in range(N_CORES):
        t = np.concatenate([tables[HPC * c + h] for h in range(HPC)], axis=1)
        in_maps.append({"tbl": np.ascontiguousarray(t)})

    nc = _get_nc()
    res = run_bass_kernel_spmd(nc, in_maps, list(range(N_CORES)))
    out = np.concatenate([res.results[c]["out"] for c in range(N_CORES)], axis=0)
    return out


def make_in_maps(slopes: np.ndarray):
    """For test harnesses: the per-core input maps this kernel uses."""
    tables = _tables(np.asarray(slopes, dtype=np.float32))
    return [
        {
            "tbl": np.ascontiguousarray(
                np.concatenate([tables[HPC * c + h] for h in range(HPC)], axis=1)
            )
        }
        for c in range(N_CORES)
    ]
